# revision 3
# baseline (speedup 1.0000x reference)
"""ActiveShiftLayer Trainium2 kernel (v2: fp16 end-to-end).

out[n,c,h,w] = bilinear sample of x[n,c, h+alpha_c, w+beta_c], zero outside
the spatial extent.

alpha,beta in [-1,1) => floor in {-1,0}, so the bilinear sample is a
separable 3-tap convolution along H then W with per-channel tap weights:
    vt[h,w]  = sum_dy wv[c,dy] * x[h+dy, w]      (dy in {-1,0,1}, zero pad)
    out[h,w] = sum_dx wh[c,dx] * vt[h, w+dx]     (dx in {-1,0,1}, zero pad)
Tap weights are computed on host from shift_param [C,2] and passed as tiny
extra inputs.

Sharding: data-parallel over batch (N=32 -> 4 per core), each core also
splits C=256 into two partition blocks -> 8 tiles of [128 channels
(partitions), 56x56 plane (free dim)] per core. Pure SPMD, no collectives.

v2 vs v1: the whole pipeline runs in fp16 (input staged fp16 on host ->
halves HBM read traffic; rel err from fp16 quantization ~3e-4 vs the 2e-2
gate). The engine-limiting H-stage is restructured around the DVE perf-mode
rules (0.96 GHz; fp16 tensor ops reach 2 elem/cycle/lane only when every
tensor AP is step +-1 AND 4B-aligned):
- V-stage on TensorE: per 512-col chunk, 3 accumulating fp16 matmuls with
  diag(wv_tap) stationary, X shifted by -56/0/+56 (zero guard rows) -> PSUM.
- ScalarE copies PSUM -> A[1+i] = vt[i] fp16 at an ODD element offset
  (ScalarE is alignment-agnostic). Both +-1 H-tap reads then land 4B-aligned
  in A: left tap reads A[i], right tap reads A[i+2].
- center tap: alternates between ScalarE (act scale from PSUM) and DVE
  (tensor_scalar from A, 2-port mode) to balance the two engines.
- H-taps on DVE: FLAT contiguous fp16 STTs at 2x (no 2D APs); row-wrap
  errors (col 0 / col 55 picking up the adjacent row) fixed by two tiny
  strided STTs per tile with negated weights.
- OUT is accumulated in fp16 and stored as-is via SWDGE (no cast needed).

Measured on trn2 (8 cores): see HW exec prints; DMA roofline for the
~12.9 MB/core of fp16-in/fp16-out traffic is ~36 us.
"""

import os
import numpy as np

N, C, H, W = 32, 256, 56, 56
NCORES = 8
NSH = N // NCORES  # batches per core
P = 128
CB = C // P        # channel blocks
HW = H * W         # 3136
CHUNK = 512        # one PSUM bank of f32 per matmul
XLEN = W + HW + W + 16  # guard row + plane + guard row + pad (fp16 elems)
ALEN = HW + 2      # vt at offset 1 with zero guards at 0 and HW+1
# center-tap engine pattern over pieces: 's' ScalarE, 'd' DVE
CENTER_PAT = os.environ.get("ASL_CENTER", "sdsds")

_CACHE = {}


def _build_nc():
    import concourse.bacc as bacc
    import concourse.mybir as mybir
    import concourse.tile as tile

    f16 = mybir.dt.float16
    f32 = mybir.dt.float32
    mult = mybir.AluOpType.mult
    add = mybir.AluOpType.add
    act_copy = mybir.ActivationFunctionType.Copy

    nc = bacc.Bacc()
    xs = nc.dram_tensor("xs", [NSH, C, H, W], f16, kind="ExternalInput")
    # wd[cb] = [diag(wv_m1) | diag(wv_0) | diag(wv_p1)] for channel block cb
    wd = nc.dram_tensor("wd", [CB, P, 3 * P], f16, kind="ExternalInput")
    # wv[cb] columns: [wh_m1, wh_0, wh_p1, -wh_m1, -wh_p1]
    wv = nc.dram_tensor("wv", [CB, P, 5], f32, kind="ExternalInput")
    ys = nc.dram_tensor("ys", [NSH, C, H, W], f16, kind="ExternalOutput")

    with tile.TileContext(nc) as tc:
        with tc.tile_pool(name="wp", bufs=1) as wp, \
             tc.tile_pool(name="xp", bufs=4) as xpool, \
             tc.tile_pool(name="ap", bufs=3) as apool, \
             tc.tile_pool(name="op", bufs=3) as opool, \
             tc.tile_pool(name="ps", bufs=2, space="PSUM") as ppool:

            wdt = []
            wvt = []

            def load_weights(cb):
                t = wp.tile([P, 3 * P], f16, tag=f"wd{cb}")
                nc.sync.dma_start(t[:], wd[cb])
                wdt.append(t)
                v = wp.tile([P, 5], f32, tag=f"wv{cb}")
                nc.sync.dma_start(v[:], wv[cb])
                wvt.append(v)

            load_weights(0)

            tiles = [(n, cb) for n in range(NSH) for cb in range(CB)]
            NT = len(tiles)

            # row boundaries of the compute pieces per tile: small pieces on
            # the first/last tile for fast pipeline fill/drain, halves
            # otherwise
            def bounds(idx):
                if idx == 0:
                    return [0, 4, 9, 16, 28, 42, 56]
                if idx == NT - 1:
                    return [0, 14, 28, 42, 49, 56]
                return [0, 28, 56]

            xtiles = {}

            def issue_load(idx):
                # loads issued ahead of compute (sync HWDGE queue holds only
                # loads, so nothing head-of-line blocks them); segments are
                # split at piece boundaries (+1 row of V-tap halo) so piece i
                # only depends on segments 0..i
                ln, lcb = tiles[idx]
                lcs = slice(lcb * P, (lcb + 1) * P)
                X = xpool.tile([P, XLEN], f16, tag="X")
                # zero guard rows above and below the plane for V-stage taps
                nc.gpsimd.memset(X[:, 0:W].bitcast(f32), 0.0)
                nc.gpsimd.memset(X[:, W + HW:W + HW + W].bitcast(f32), 0.0)
                xflat = xs[ln, lcs, :, :].rearrange("p h w -> p (h w)")
                b = bounds(idx)
                cuts = [min(r + 1, H) for r in b[1:-1]] + [H]
                r0 = 0
                for r1 in cuts:
                    nc.sync.dma_start(X[:, W + r0 * W:W + r1 * W],
                                      xflat[:, r0 * W:r1 * W])
                    r0 = r1
                xtiles[idx] = X

            issue_load(0)
            load_weights(1)
            issue_load(1)
            issue_load(2)

            piece_no = 0  # global piece counter for the center-engine pattern

            for tidx, (n, cb) in enumerate(tiles):
                wvc = wvt[cb]
                cs = slice(cb * P, (cb + 1) * P)
                if tidx + 3 < len(tiles):
                    issue_load(tidx + 3)
                X = xtiles.pop(tidx)

                A = apool.tile([P, ALEN], f16)
                # zero guards at A[0] (left tap of plane elem 0) and
                # A[HW+1] (right tap of plane elem HW-1); memsets cover a
                # 4B pair, the interior element is overwritten by the
                # PSUM->A copies below (program order keeps this correct)
                nc.gpsimd.memset(A[:, 0:2].bitcast(f32), 0.0)
                nc.gpsimd.memset(A[:, HW:HW + 2].bitcast(f32), 0.0)
                OUT = opool.tile([P, HW], f16)

                def right_stt(p0, PZ):
                    # out[i] += wh_p1 * vt[i+1]; vt[i+1] = A[i+2] (aligned)
                    nc.vector.scalar_tensor_tensor(
                        OUT[:, p0:p0 + PZ], A[:, p0 + 2:p0 + 2 + PZ],
                        wvc[:, 2:3], OUT[:, p0:p0 + PZ], op0=mult, op1=add)

                tb = bounds(tidx)
                prev = None
                for rr0, rr1 in zip(tb[:-1], tb[1:]):
                    p0 = rr0 * W
                    PZ = (rr1 - rr0) * W

                    # V-stage on TensorE: accumulating diag matmuls, taps at
                    # row offsets -56/0/+56 into guarded X
                    PS = ppool.tile([P, 4 * CHUNK], f32, tag="ps")
                    for c0 in range(0, PZ, CHUNK):
                        cn = min(CHUNK, PZ - c0)
                        for tap in range(3):
                            o = W + p0 + c0 + (tap - 1) * W
                            nc.tensor.matmul(
                                PS[:, c0:c0 + cn],
                                wdt[cb][:, tap * P:(tap + 1) * P],
                                X[:, o:o + cn],
                                start=(tap == 0), stop=(tap == 2))

                    # vt -> SBUF at odd offset: A[1+i] = vt[i]
                    nc.scalar.activation(A[:, 1 + p0:1 + p0 + PZ],
                                         PS[:, 0:PZ], act_copy)

                    # center tap: OUT = wh_0 * vt
                    if CENTER_PAT[piece_no % len(CENTER_PAT)] == 's':
                        nc.scalar.activation(OUT[:, p0:p0 + PZ], PS[:, 0:PZ],
                                             act_copy, scale=wvc[:, 1:2])
                    else:
                        nc.vector.tensor_scalar_mul(
                            OUT[:, p0:p0 + PZ], A[:, 1 + p0:1 + p0 + PZ],
                            wvc[:, 1:2])
                    piece_no += 1

                    # left tap: out[i] += wh_m1 * vt[i-1]; vt[i-1] = A[i]
                    nc.vector.scalar_tensor_tensor(
                        OUT[:, p0:p0 + PZ], A[:, p0:p0 + PZ], wvc[:, 0:1],
                        OUT[:, p0:p0 + PZ], op0=mult, op1=add)

                    # right tap of the PREVIOUS piece: its last element reads
                    # A[1+p0] of THIS piece, so it is deferred until this
                    # piece's PSUM->A copy has been issued
                    if prev is not None:
                        right_stt(*prev)
                    prev = (p0, PZ)

                right_stt(*prev)  # last piece: A[HW+1] guard is zero

                # row-wrap fixups for the flat H-taps: col 0 wrongly picked
                # up wh_m1 * vt[h-1, 55] (A[h*56]), col 55 wrongly picked up
                # wh_p1 * vt[h+1, 0] (A[57 + h*56]); subtract via negated
                # weights. Row h=0 / h=55 read the zero guards -> no-ops.
                o2 = OUT[:, 0:HW].rearrange("p (h w) -> p h w", w=W)
                am = A[:, 0:HW].rearrange("p (h w) -> p h w", w=W)[:, :, 0]
                ap1 = A[:, 2:2 + HW].rearrange("p (h w) -> p h w",
                                               w=W)[:, :, W - 1]
                nc.vector.scalar_tensor_tensor(
                    o2[:, :, 0], am, wvc[:, 3:4], o2[:, :, 0],
                    op0=mult, op1=add)
                nc.vector.scalar_tensor_tensor(
                    o2[:, :, W - 1], ap1, wvc[:, 4:5], o2[:, :, W - 1],
                    op0=mult, op1=add)

                # whole-tile store (fp16, no cast): SWDGE keeps the sync
                # HWDGE queue free for loads
                nc.gpsimd.dma_start(
                    ys[n, cs, :, :].rearrange("p h w -> p (h w)"), OUT[:])
    nc.finalize()
    return nc


def _tap_weights(shift):
    """Per-channel 3-tap weights over offsets {-1,0,1} for shift in [-1,1)."""
    f = np.floor(shift)
    t = (shift - f).astype(np.float32)
    assert np.all((f == -1) | (f == 0)), "shift outside [-1,1) unsupported"
    w_m1 = np.where(f == -1, 1 - t, 0).astype(np.float32)
    w_0 = np.where(f == -1, t, 1 - t).astype(np.float32)
    w_p1 = np.where(f == 0, t, 0).astype(np.float32)
    return w_m1, w_0, w_p1


def _host_weights(sp):
    wh_m1, wh_0, wh_p1 = _tap_weights(sp[:, 1])  # beta: W shift
    wv_m1, wv_0, wv_p1 = _tap_weights(sp[:, 0])  # alpha: H shift
    # V-stage diag matrices, fp16. Layout [CB, P, 3*P] matches the SBUF
    # weight tile exactly (contiguous per-partition DMA).
    wd = np.zeros((CB, 3, P, P), np.float32)
    for cb in range(CB):
        cs = slice(cb * P, (cb + 1) * P)
        for t, w in enumerate((wv_m1, wv_0, wv_p1)):
            wd[cb, t] = np.diag(w[cs])
    wd = wd.transpose(0, 2, 1, 3).reshape(CB, P, 3 * P).astype(np.float16)
    # H-stage per-partition scalars + negated outer taps for wrap fixups
    wv = np.stack([wh_m1, wh_0, wh_p1, -wh_m1, -wh_p1], axis=1)
    wv = np.ascontiguousarray(wv.astype(np.float32).reshape(CB, P, 5))
    return np.ascontiguousarray(wd), wv


def _install_trace_shim():
    """Dev-only: register the NTFF profile hook this container's antenv lacks,
    and stub out the artifact upload (zero-egress container)."""
    import sys
    import types

    try:
        from antenv.axon_hooks import get_axon_ntff_profile_hook  # noqa: F401
    except ImportError:
        from trn_agent_boot.trn_boot import _ntff_profile_via_ctypes

        hook = _ntff_profile_via_ctypes("/opt/axon/libaxon_pjrt.so")
        mod = types.ModuleType("antenv.axon_hooks")
        mod.get_axon_ntff_profile_hook = lambda: hook
        mod.set_axon_ntff_profile_hook = lambda h: None
        import antenv

        sys.modules["antenv.axon_hooks"] = mod
        antenv.axon_hooks = mod

    import concourse.bass_utils as bu

    bu.upload_artifacts = lambda tmpdir: tmpdir


def kernel(x, shift_param):
    from concourse.bass_utils import run_bass_kernel_spmd

    x = np.asarray(x)
    sp = np.asarray(shift_param, dtype=np.float32)
    assert x.shape == (N, C, H, W)
    x16 = np.ascontiguousarray(x.astype(np.float16))

    wd, wv = _host_weights(sp)

    if "nc" not in _CACHE:
        _CACHE["nc"] = _build_nc()
    nc = _CACHE["nc"]

    in_maps = [{"xs": x16[i * NSH:(i + 1) * NSH], "wd": wd, "wv": wv}
               for i in range(NCORES)]
    trace = os.environ.get("ASL_TRACE") == "1"
    if trace:
        _install_trace_shim()
    res = run_bass_kernel_spmd(nc, in_maps, list(range(NCORES)), trace=trace)
    if trace:
        print(f"HW exec time: {res.exec_time_ns} ns")
        _CACHE["last_result"] = res
    out = np.concatenate([r["ys"] for r in res.results], axis=0)
    return out.astype(np.float32)


# revision 5
# speedup vs baseline: 1.2282x; 1.2282x over previous
"""ActiveShiftLayer Trainium2 kernel (v3: fp16, 3-engine-balanced H-stage).

out[n,c,h,w] = bilinear sample of x[n,c, h+alpha_c, w+beta_c], zero outside
the spatial extent.

alpha,beta in [-1,1) => floor in {-1,0}, so the bilinear sample is a
separable 3-tap convolution along H then W. KEY: the two outer taps are
EXCLUSIVE per channel (floor -1 -> only the -1 tap, floor 0 -> only the +1
tap), so each stage is really a 2-tap with a per-channel side.

Sharding: data-parallel over batch (N=32 -> 4 per core) x 2 channel blocks
of 128. Channels are HOST-SORTED by the sign of the W-shift so blocks are
(nearly) side-pure -> fewer side passes; host un-permutes the output.

Per-core engine budget (measured rates, 25088 elems/partition total):
- TensorE  (0.42 ns/col warm): V-stage 3 diag-matmul taps per 512-chunk
  (PSUM f32) + the H-stage for a subset of pieces (center + per-need side
  diags at flat offsets -1/0/+1 into the vt buffer).
- ScalarE  (0.945 us per 784-elem piece): PSUM->SBUF copies. copy1 writes
  vt into A at element offset 1 (A[1+i] = vt[i]) so both +-1 taps read A
  4B-aligned; copy2 drains the PE-H PSUM to OUT.
- VectorE  (STT 1x, tensor_scalar 4x): H-stage for the remaining pieces:
  center = tensor_scalar_mul from A, sides = flat fp16 STTs (aligned via
  the odd-offset A layout).
- GpSimd: SWDGE stores, guards, and the tiny strided row-wrap fixup STTs
  (flat +-1 taps wrap at row edges; subtract with negated weights).
All fp16 end-to-end (input staged fp16 on host -> halves HBM read traffic;
total quantization ~4e-4 vs the 2e-2 gate).

Pieces are quarter-planes (14 rows = 784 elems) so PSUM tiles are 2 banks
-> a 4-deep ring serves both V and PE-H stages. H work for piece k is
deferred until copy1(k+1) (the +1 tap's last element crosses the piece
boundary).
"""

import os
import numpy as np

N, C, H, W = 32, 256, 56, 56
NCORES = 8
NSH = N // NCORES  # batches per core
P = 128
CB = C // P        # channel blocks
HW = H * W         # 3136
CHUNK = 512
XLEN = W + HW + W + 16  # guard row + plane + guard row + pad (fp16 elems)
ALEN = HW + 2      # vt at offset 1 with zero guards at 0 and HW+1
PSLEN = 1024       # 2 PSUM banks; pieces are <= 784 elems

# H-stage engine pattern per channel block, cycled over pieces:
# 'p' = TensorE (matmuls + scalar copy2), 'd' = VectorE (TS center + STTs)
HPAT = (os.environ.get("ASL_HPAT_B0", "pdd"),
        os.environ.get("ASL_HPAT_B1", "dpdd"))
FIXUP_ENGINE = os.environ.get("ASL_FIXUP", "vector")

_CACHE = {}


def _build_nc(flags):
    # flags[cb] = (needL, needR, needVm, needVp)
    import concourse.bacc as bacc
    import concourse.mybir as mybir
    import concourse.tile as tile

    f16 = mybir.dt.float16
    f32 = mybir.dt.float32
    mult = mybir.AluOpType.mult
    add = mybir.AluOpType.add
    act_copy = mybir.ActivationFunctionType.Copy

    nc = bacc.Bacc()
    xs = nc.dram_tensor("xs", [NSH, C, H, W], f16, kind="ExternalInput")
    # wd[cb] = [diag(wv_m1)|diag(wv_0)|diag(wv_p1)|diag(wh_m1)|diag(wh_0)|
    #           diag(wh_p1)] for (sorted) channel block cb
    wd = nc.dram_tensor("wd", [CB, P, 6 * P], f16, kind="ExternalInput")
    # wv[cb] columns: [wh_m1, wh_0, wh_p1, -wh_m1, -wh_p1]
    wv = nc.dram_tensor("wv", [CB, P, 5], f32, kind="ExternalInput")
    ys = nc.dram_tensor("ys", [NSH, C, H, W], f16, kind="ExternalOutput")

    fix_eng_attr = "gpsimd" if FIXUP_ENGINE == "gpsimd" else "vector"

    with tile.TileContext(nc) as tc:
        with tc.tile_pool(name="wp", bufs=1) as wp, \
             tc.tile_pool(name="xp", bufs=4) as xpool, \
             tc.tile_pool(name="ap", bufs=3) as apool, \
             tc.tile_pool(name="op", bufs=3) as opool, \
             tc.tile_pool(name="ps", bufs=4, space="PSUM") as ppool:

            fix_eng = getattr(nc, fix_eng_attr)
            wdt = []
            wvt = []

            def load_weights(cb):
                t = wp.tile([P, 6 * P], f16, tag=f"wd{cb}")
                nc.sync.dma_start(t[:], wd[cb])
                wdt.append(t)
                v = wp.tile([P, 5], f32, tag=f"wv{cb}")
                nc.sync.dma_start(v[:], wv[cb])
                wvt.append(v)

            load_weights(0)

            tiles = [(n, cb) for n in range(NSH) for cb in range(CB)]
            NT = len(tiles)

            def bounds(idx):
                if idx == 0:
                    return [0, 4, 9, 16, 28, 42, 56]
                if idx == NT - 1:
                    return [0, 14, 28, 42, 49, 56]
                return [0, 14, 28, 42, 56]

            xtiles = {}

            def issue_load(idx):
                ln, lcb = tiles[idx]
                lcs = slice(lcb * P, (lcb + 1) * P)
                X = xpool.tile([P, XLEN], f16, tag="X")
                nc.gpsimd.memset(X[:, 0:W].bitcast(f32), 0.0)
                nc.gpsimd.memset(X[:, W + HW:W + HW + W].bitcast(f32), 0.0)
                xflat = xs[ln, lcs, :, :].rearrange("p h w -> p (h w)")
                b = bounds(idx)
                # load in two segments (piece halo +1 row) so early pieces
                # can start before the whole tile lands
                cuts = [min(b[len(b) // 2] + 1, H), H]
                r0 = 0
                for r1 in cuts:
                    if r1 > r0:
                        nc.sync.dma_start(X[:, W + r0 * W:W + r1 * W],
                                          xflat[:, r0 * W:r1 * W])
                    r0 = r1
                xtiles[idx] = X

            issue_load(0)
            load_weights(1)
            issue_load(1)
            issue_load(2)

            pcnt = [0, 0]  # per-block piece counter for the H pattern

            for tidx, (n, cb) in enumerate(tiles):
                wvc = wvt[cb]
                needL, needR, needVm, needVp = flags[cb]
                cs = slice(cb * P, (cb + 1) * P)
                if tidx + 3 < len(tiles):
                    issue_load(tidx + 3)
                X = xtiles.pop(tidx)

                A = apool.tile([P, ALEN], f16)
                nc.gpsimd.memset(A[:, 0:2].bitcast(f32), 0.0)
                nc.gpsimd.memset(A[:, HW:HW + 2].bitcast(f32), 0.0)
                OUT = opool.tile([P, HW], f16)

                vtaps = [t for t, need in enumerate(
                    (needVm, True, needVp)) if need]
                htaps = [t for t, need in enumerate(
                    (needL, True, needR)) if need]

                def emit_h(p0, PZ):
                    on_pe = HPAT[cb][pcnt[cb] % len(HPAT[cb])] == 'p'
                    pcnt[cb] += 1
                    if on_pe:
                        PSB = ppool.tile([P, PSLEN], f32, tag="ps")
                        for c0 in range(0, PZ, CHUNK):
                            cn = min(CHUNK, PZ - c0)
                            for i, tap in enumerate(htaps):
                                o = p0 + c0 + tap  # A[1+i+dx] = A[i+tap]
                                nc.tensor.matmul(
                                    PSB[:, c0:c0 + cn],
                                    wdt[cb][:, (3 + tap) * P:(4 + tap) * P],
                                    A[:, o:o + cn],
                                    start=(i == 0), stop=(i == len(htaps) - 1))
                        nc.scalar.activation(OUT[:, p0:p0 + PZ],
                                             PSB[:, 0:PZ], act_copy)
                    else:
                        # center: OUT = wh_0 * vt
                        nc.vector.tensor_scalar_mul(
                            OUT[:, p0:p0 + PZ], A[:, 1 + p0:1 + p0 + PZ],
                            wvc[:, 1:2])
                        if needL:  # out[i] += wh_m1 * vt[i-1] = A[i]
                            nc.vector.scalar_tensor_tensor(
                                OUT[:, p0:p0 + PZ], A[:, p0:p0 + PZ],
                                wvc[:, 0:1], OUT[:, p0:p0 + PZ],
                                op0=mult, op1=add)
                        if needR:  # out[i] += wh_p1 * vt[i+1] = A[i+2]
                            nc.vector.scalar_tensor_tensor(
                                OUT[:, p0:p0 + PZ], A[:, p0 + 2:p0 + 2 + PZ],
                                wvc[:, 2:3], OUT[:, p0:p0 + PZ],
                                op0=mult, op1=add)

                tb = bounds(tidx)
                prev = None
                for rr0, rr1 in zip(tb[:-1], tb[1:]):
                    p0 = rr0 * W
                    PZ = (rr1 - rr0) * W

                    # V-stage on TensorE
                    PSA = ppool.tile([P, PSLEN], f32, tag="ps")
                    for c0 in range(0, PZ, CHUNK):
                        cn = min(CHUNK, PZ - c0)
                        for i, tap in enumerate(vtaps):
                            o = W + p0 + c0 + (tap - 1) * W
                            nc.tensor.matmul(
                                PSA[:, c0:c0 + cn],
                                wdt[cb][:, tap * P:(tap + 1) * P],
                                X[:, o:o + cn],
                                start=(i == 0), stop=(i == len(vtaps) - 1))

                    # copy1: vt -> A at odd offset (fp16)
                    nc.scalar.activation(A[:, 1 + p0:1 + p0 + PZ],
                                         PSA[:, 0:PZ], act_copy)

                    # H of the PREVIOUS piece (its +1 tap needs this piece's
                    # first vt element)
                    if prev is not None:
                        emit_h(*prev)
                    prev = (p0, PZ)

                emit_h(*prev)  # last piece: A[HW+1] guard is zero

                # row-wrap fixups for the flat +-1 taps (subtract the
                # wrapped contribution with negated weights); rows 0/55
                # read the zero guards -> no-ops
                o2 = OUT[:, 0:HW].rearrange("p (h w) -> p h w", w=W)
                if needL:
                    am = A[:, 0:HW].rearrange("p (h w) -> p h w", w=W)[:, :, 0]
                    fix_eng.scalar_tensor_tensor(
                        o2[:, :, 0], am, wvc[:, 3:4], o2[:, :, 0],
                        op0=mult, op1=add)
                if needR:
                    ap1 = A[:, 2:2 + HW].rearrange("p (h w) -> p h w",
                                                   w=W)[:, :, W - 1]
                    fix_eng.scalar_tensor_tensor(
                        o2[:, :, W - 1], ap1, wvc[:, 4:5], o2[:, :, W - 1],
                        op0=mult, op1=add)

                nc.gpsimd.dma_start(
                    ys[n, cs, :, :].rearrange("p h w -> p (h w)"), OUT[:])
    nc.finalize()
    return nc


def _tap_weights(shift):
    """Per-channel 3-tap weights over offsets {-1,0,1} for shift in [-1,1)."""
    f = np.floor(shift)
    t = (shift - f).astype(np.float32)
    assert np.all((f == -1) | (f == 0)), "shift outside [-1,1) unsupported"
    w_m1 = np.where(f == -1, 1 - t, 0).astype(np.float32)
    w_0 = np.where(f == -1, t, 1 - t).astype(np.float32)
    w_p1 = np.where(f == 0, t, 0).astype(np.float32)
    return w_m1, w_0, w_p1


def _host_prep(sp):
    """Channel sort by W-shift side + weight tensors (sorted order)."""
    beta_side = (np.floor(sp[:, 1]) == 0).astype(np.int32)  # 0=left, 1=right
    perm = np.argsort(beta_side, kind="stable")
    sps = sp[perm]
    wh_m1, wh_0, wh_p1 = _tap_weights(sps[:, 1])  # beta: W shift
    wv_m1, wv_0, wv_p1 = _tap_weights(sps[:, 0])  # alpha: H shift

    flags = []
    for cb in range(CB):
        cs = slice(cb * P, (cb + 1) * P)
        flags.append((bool(wh_m1[cs].any()), bool(wh_p1[cs].any()),
                      bool(wv_m1[cs].any()), bool(wv_p1[cs].any())))

    wd = np.zeros((CB, 6, P, P), np.float32)
    for cb in range(CB):
        cs = slice(cb * P, (cb + 1) * P)
        for t, w in enumerate((wv_m1, wv_0, wv_p1, wh_m1, wh_0, wh_p1)):
            wd[cb, t] = np.diag(w[cs])
    wd = wd.transpose(0, 2, 1, 3).reshape(CB, P, 6 * P).astype(np.float16)
    wvv = np.stack([wh_m1, wh_0, wh_p1, -wh_m1, -wh_p1], axis=1)
    wvv = np.ascontiguousarray(wvv.astype(np.float32).reshape(CB, P, 5))
    return perm, tuple(flags), np.ascontiguousarray(wd), wvv


def _install_trace_shim():
    """Dev-only: register the NTFF profile hook this container's antenv lacks,
    and stub out the artifact upload (zero-egress container)."""
    import sys
    import types

    try:
        from antenv.axon_hooks import get_axon_ntff_profile_hook  # noqa: F401
    except ImportError:
        from trn_agent_boot.trn_boot import _ntff_profile_via_ctypes

        hook = _ntff_profile_via_ctypes("/opt/axon/libaxon_pjrt.so")
        mod = types.ModuleType("antenv.axon_hooks")
        mod.get_axon_ntff_profile_hook = lambda: hook
        mod.set_axon_ntff_profile_hook = lambda h: None
        import antenv

        sys.modules["antenv.axon_hooks"] = mod
        antenv.axon_hooks = mod

    import concourse.bass_utils as bu

    bu.upload_artifacts = lambda tmpdir: tmpdir


def kernel(x, shift_param):
    from concourse.bass_utils import run_bass_kernel_spmd

    x = np.asarray(x)
    sp = np.asarray(shift_param, dtype=np.float32)
    assert x.shape == (N, C, H, W)

    perm, flags, wd, wv = _host_prep(sp)
    x16 = np.ascontiguousarray(x[:, perm].astype(np.float16))

    key = ("nc", flags)
    if key not in _CACHE:
        _CACHE[key] = _build_nc(flags)
    nc = _CACHE[key]

    in_maps = [{"xs": x16[i * NSH:(i + 1) * NSH], "wd": wd, "wv": wv}
               for i in range(NCORES)]
    trace = os.environ.get("ASL_TRACE") == "1"
    if trace:
        _install_trace_shim()
    res = run_bass_kernel_spmd(nc, in_maps, list(range(NCORES)), trace=trace)
    if trace:
        print(f"HW exec time: {res.exec_time_ns} ns")
        _CACHE["last_result"] = res
    ys = np.concatenate([r["ys"] for r in res.results], axis=0)
    out = np.empty((N, C, H, W), np.float32)
    out[:, perm] = ys.astype(np.float32)
    return out
